# revision 1
# baseline (speedup 1.0000x reference)
"""AttentionWithPairBias Trainium2 kernel, 8-way sequence-parallel over query rows.

Strategy:
  - Each of the 8 cores owns 96 of the 768 query rows i.
  - The dominant work is the pair-bias reduction: pair [768,768,128] is
    host-transposed per core to [z=128, ij=96*768] so the z-contraction maps
    onto the TensorE partition axis. LayerNorm over z is algebraically folded:
        LN(z) @ (gz*Wb)  =  rsig_ij * (z @ W'')        (+ const_h, softmax-invariant)
    with W'' = gz*Wb - colsum(gz*Wb)/128.  mu and E[z^2] come out of the same
    matmuls via extra ones/128 columns; the squared stream is produced on
    ScalarE.  Four i-rows are packed per PSUM bank (partition offsets 0/32/64/96
    via zero-padded stationary operands) so the PSUM->SBUF copy runs with full
    partition utilization.  Per-(i,j) rsig is applied after a partition-remap
    SBUF->SBUF DMA puts the bias into [i, h, j] layout.
  - q/k/v/gate projections, attention, softmax (no max-subtraction: logits are
    O(6)), AV, and the output projection run per-core on its 96 rows.
  - All matmuls use float32r (full-rate PE, ~1e-3 rel precision).
"""
import sys

sys.path.insert(0, "/opt/trn_rl_repo")

import numpy as np

import concourse.bacc as bacc
import concourse.tile as tile
from concourse import mybir
from concourse.bass_utils import run_bass_kernel_spmd

from contextlib import ExitStack

F32 = mybir.dt.float32
F32R = mybir.dt.float32r
BF16 = mybir.dt.bfloat16

PAIR_BF16 = True   # pair stream + bias roundtrip in bf16 (halves dominant DMA traffic)

L = 768
CS = 384
CZ = 128
H = 8
HD = 48
HP = 64          # padded head stride in permuted c2 layout
CP = H * HP      # 512, padded c2 size for q/k/v
NCORES = 8
LC = L // NCORES  # 96 rows per core
EPS = 1e-5
NQUAD = LC // 4   # 24 quads of 4 i-rows
JH = L // 2       # 384, half of j


def build(n_iter=1):
    nc = bacc.Bacc("TRN2", target_bir_lowering=False, debug=False, num_devices=NCORES)

    ZDT = BF16 if PAIR_BF16 else F32R
    SDT = BF16 if PAIR_BF16 else F32
    pairT_d = nc.declare_dram_parameter("pairT", [CZ, LC * L], ZDT, isOutput=False)
    sing_d = nc.declare_dram_parameter("sing", [L, CS], F32, isOutput=False)
    sown_d = nc.declare_dram_parameter("sown", [LC, CS], F32, isOutput=False)
    wzs_d = nc.declare_dram_parameter("wzs", [CZ, 2, 4, 106], ZDT, isOutput=False)
    wqkv_d = nc.declare_dram_parameter("wqkv", [CS, 3, CP], F32R, isOutput=False)
    wgt_d = nc.declare_dram_parameter("wgt", [CS, CS], F32R, isOutput=False)
    wot_d = nc.declare_dram_parameter("wot", [HD, H, CS], F32R, isOutput=False)
    qbkb_d = nc.declare_dram_parameter("qbkb", [128, 8], F32, isOutput=False)
    bb_d = nc.declare_dram_parameter("bb", [CP + 2 * CS], F32, isOutput=False)
    ident_d = nc.declare_dram_parameter("ident", [128, 128], F32R, isOutput=False)
    identb_d = nc.declare_dram_parameter("identb", [LC, LC], BF16, isOutput=False)
    y_d = nc.declare_dram_parameter("y", [LC, CS], F32, isOutput=True)
    drs_d = nc.dram_tensor("drs", [2 * NQUAD, 106, JH], SDT)  # staged-unit scratch

    pairT3 = pairT_d[:].rearrange("z (i j) -> z i j", j=L)

    with tile.TileContext(nc) as tc, ExitStack() as ctx:
        singles = ctx.enter_context(tc.tile_pool(name="singles", bufs=1))
        persist = ctx.enter_context(tc.tile_pool(name="persist", bufs=1))
        arena = ctx.enter_context(tc.tile_pool(name="arena", bufs=1))
        import os
        _sb = int(os.environ.get("STREAM_BUFS", "5"))
        _zb = int(os.environ.get("Z_BUFS", "5"))
        _ub = int(os.environ.get("U_BUFS", "3"))
        _wb = int(os.environ.get("W_BUFS", "3"))
        stream = ctx.enter_context(tc.tile_pool(name="stream", bufs=_sb))
        once = ctx.enter_context(tc.tile_pool(name="once", bufs=1))
        pstream = ctx.enter_context(tc.tile_pool(name="pstream", bufs=3))
        zpool = ctx.enter_context(tc.tile_pool(name="zpool", bufs=_zb))
        small = ctx.enter_context(tc.tile_pool(name="small", bufs=4))
        pp_u = ctx.enter_context(tc.tile_pool(name="pp_u", bufs=_ub, space="PSUM"))
        pp_tp = ctx.enter_context(tc.tile_pool(name="pp_tp", bufs=2, space="PSUM"))
        pp_work = ctx.enter_context(tc.tile_pool(name="pp_work", bufs=_wb, space="PSUM"))

        # ---- constants / weights ----
        ident = singles.tile([128, 128], F32R)
        nc.scalar.dma_start(out=ident, in_=ident_d[:])
        identb = singles.tile([LC, LC], BF16)
        nc.scalar.dma_start(out=identb, in_=identb_d[:])
        wzs_sb = singles.tile([CZ, 2, 4, 106], ZDT)
        nc.scalar.dma_start(out=wzs_sb, in_=wzs_d[:])
        wraw_sb = wzs_sb[:, 0]
        wsq_sb = wzs_sb[:, 1]
        wqkv_sb = singles.tile([128, 3, 3, CP], F32R)
        nc.scalar.dma_start(out=wqkv_sb, in_=wqkv_d[:].rearrange("(b p) w n -> p b w n", p=128))
        wgt_sb = singles.tile([128, 3, CS], F32R)
        nc.scalar.dma_start(out=wgt_sb, in_=wgt_d[:].rearrange("(b p) n -> p b n", p=128))
        wot_sb = singles.tile([HD, H, CS], F32R)
        nc.scalar.dma_start(out=wot_sb, in_=wot_d[:])
        qbkb_sb = singles.tile([128, 8], F32)
        nc.scalar.dma_start(out=qbkb_sb, in_=qbkb_d[:])
        bb_sb = singles.tile([128, CP + 2 * CS], F32)
        import concourse.bass as bass
        _bb = bb_d[:]
        nc.scalar.dma_start(out=bb_sb, in_=bass.AP(tensor=_bb.tensor, offset=_bb.offset,
                                                   ap=[[0, 128]] + _bb.ap))
        vb_bc = bb_sb[:, 0:CP]
        gb_bc = bb_sb[:, CP : CP + CS]
        bo_bc = bb_sb[:, CP + CS : CP + 2 * CS]
        eps128 = singles.tile([128, 1], F32)
        nc.vector.memset(eps128, EPS)

        def emit_iter():
            # ---- pair-bias stream ----
            bias_hij = arena.tile([LC, 10, L], SDT, tag="big")  # h=0..7 bias, 8=mu, 9=ex2
            rsig = persist.tile([LC, L], F32)

            def gather_wave(u0, u1, eng):
                # gather units [u0, u1) = i-rows [2*u0, 2*u1) from drs, then
                # stats -> rsig and scale this wave's bias rows in place.
                # Row starts must be 32-aligned for the engine ops below.
                r0, nr = 2 * u0, 2 * (u1 - u0)
                drs_w = drs_d[u0:u1]
                drs_v = drs_w[:, 0:96].rearrange("(Q hf) (q hh) j -> q hf Q hh j", hf=2, q=3)
                drs_v3 = drs_w[:, 96:106].rearrange("(Q hf) hh j -> hf Q hh j", hf=2)
                bias_w = bias_hij[r0 : r0 + nr, :, :]
                bias_v = bias_w.rearrange("(Q q) h (hf jj) -> q hf Q h jj", q=4, hf=2)
                for q in range(4):
                    for hf in range(2):
                        if q < 3:
                            eng.dma_start(out=bias_v[q, hf], in_=drs_v[q, hf, :, 0:10, :])
                        else:
                            eng.dma_start(out=bias_v[q, hf], in_=drs_v3[hf, :, :, :])
                rs = rsig[r0 : r0 + nr, :]
                mu_w = bias_w[:, 8, :]
                ex2_w = bias_w[:, 9, :]
                nc.vector.tensor_mul(out=rs, in0=mu_w, in1=mu_w)
                nc.vector.tensor_tensor(out=rs, in0=ex2_w, in1=rs,
                                        op=mybir.AluOpType.subtract)
                nc.scalar.activation(out=rs, in_=rs,
                                     func=mybir.ActivationFunctionType.Sqrt,
                                     bias=eps128[:nr])
                nc.vector.reciprocal(out=rs, in_=rs)
                meng = nc.vector if u0 == 0 else nc.gpsimd
                for h in range(H):
                    meng.tensor_mul(out=bias_w[:, h, :], in0=bias_w[:, h, :], in1=rs)

            def emit_projections():
                # ---- LayerNorm(single) ----
                s_sb = arena.tile([128, 6, CS], F32R, tag="big2")   # LN(single), i-major tiles
                so_sb = persist.tile([LC, CS], F32R)         # LN(single_own)
                x_all = once.tile([128, 6, CS], F32, tag="ln_x")
                nc.scalar.dma_start(out=x_all, in_=sing_d[:].rearrange("(t p) n -> p t n", p=128))
                sraw_sb = persist.tile([LC, CS], F32)        # raw single_own (residual)
                nc.scalar.dma_start(out=sraw_sb, in_=sown_d[:])

                def layernorm(dst, x, rows):
                    bn = small.tile([128, 6], F32, tag="ln_bn")
                    nc.vector.bn_stats(out=bn[:rows], in_=x)
                    mv = small.tile([128, 2], F32, tag="ln_mv")
                    nc.vector.bn_aggr(out=mv[:rows], in_=bn[:rows])
                    std = small.tile([128, 1], F32, tag="ln_std")
                    nc.scalar.activation(out=std[:rows], in_=mv[:rows, 1:2],
                                         func=mybir.ActivationFunctionType.Sqrt,
                                         bias=eps128[:rows])
                    rstd = small.tile([128, 1], F32, tag="ln_rstd")
                    nc.vector.reciprocal(out=rstd[:rows], in_=std[:rows])
                    nc.vector.tensor_scalar(out=dst, in0=x,
                                            scalar1=mv[:rows, 0:1], scalar2=rstd[:rows],
                                            op0=mybir.AluOpType.subtract,
                                            op1=mybir.AluOpType.mult)

                for t in range(6):
                    layernorm(s_sb[:, t, :], x_all[:, t, :], 128)
                layernorm(so_sb[:], sraw_sb[:], LC)

                # ---- transposes: sT [c1, j] and sTo [c1, own-i] ----
                sT_sb = persist.tile([128, 3, L], F32R)
                for jb in range(6):
                    for cb in range(3):
                        pt = pp_tp.tile([128, 128], F32R, tag="tp")
                        nc.tensor.transpose(pt, s_sb[:, jb, 128 * cb : 128 * (cb + 1)], ident)
                        nc.vector.tensor_copy(out=sT_sb[:, cb, 128 * jb : 128 * (jb + 1)], in_=pt)
                sTo_sb = persist.tile([128, 3, LC], F32R)
                for cb in range(3):
                    pt = pp_tp.tile([128, LC], F32R, tag="tp")
                    nc.tensor.transpose(pt, so_sb[:, 128 * cb : 128 * (cb + 1)], ident[:LC, :LC])
                    nc.vector.tensor_copy(out=sTo_sb[:, cb, :], in_=pt)

                # ---- projections ----
                qTo_sb = persist.tile([128, 4, LC], F32R)      # q^T (own rows), permuted heads
                for b in range(4):
                    ps = pp_work.tile([128, 512], F32, tag="work")
                    for kb in range(3):
                        nc.tensor.matmul(ps[:, :LC], lhsT=wqkv_sb[:, kb, 0, 128 * b : 128 * (b + 1)],
                                         rhs=sTo_sb[:, kb, :], start=(kb == 0), stop=(kb == 2))
                    nc.vector.tensor_scalar_add(out=qTo_sb[:, b, :], in0=ps[:, :LC],
                                                scalar1=qbkb_sb[:, b : b + 1])

                kT_sb = persist.tile([128, 4, L], F32R)        # k^T (all rows), permuted heads
                for b in range(4):
                    for jh in range(2):
                        ps = pp_work.tile([128, 512], F32, tag="work")
                        for kb in range(3):
                            nc.tensor.matmul(ps[:, :JH], lhsT=wqkv_sb[:, kb, 1, 128 * b : 128 * (b + 1)],
                                             rhs=sT_sb[:, kb, JH * jh : JH * (jh + 1)],
                                             start=(kb == 0), stop=(kb == 2))
                        nc.vector.tensor_scalar_add(out=kT_sb[:, b, JH * jh : JH * (jh + 1)],
                                                    in0=ps[:, :JH],
                                                    scalar1=qbkb_sb[:, 4 + b : 5 + b])

                v_sb = persist.tile([128, 6, CP], BF16)        # v (all rows), [j, c2-perm]
                for jb in range(6):
                    ps = pp_work.tile([128, 512], F32, tag="work")
                    for kb in range(3):
                        nc.tensor.matmul(ps, lhsT=sT_sb[:, kb, 128 * jb : 128 * (jb + 1)],
                                         rhs=wqkv_sb[:, kb, 2, :], start=(kb == 0), stop=(kb == 2))
                    nc.vector.tensor_add(out=v_sb[:, jb, :], in0=ps, in1=vb_bc)

                gate_sb = persist.tile([LC, CS], F32)
                psg = pp_work.tile([128, 512], F32, tag="work")
                for kb in range(3):
                    nc.tensor.matmul(psg[:LC, :CS], lhsT=sTo_sb[:, kb, :], rhs=wgt_sb[:, kb, :],
                                     start=(kb == 0), stop=(kb == 2))
                gtmp = once.tile([LC, CS], F32, tag="gtmp")
                nc.vector.tensor_add(out=gtmp, in0=psg[:LC, :CS], in1=gb_bc[:LC])
                nc.scalar.activation(out=gate_sb, in_=gtmp,
                                     func=mybir.ActivationFunctionType.Sigmoid)


                return qTo_sb, kT_sb, v_sb, gate_sb, sraw_sb

            for U in range(2 * NQUAD):
                Q, hf = U // 2, U % 2
                zt = zpool.tile([CZ, 4, JH], ZDT, tag="zt")
                nc.sync.dma_start(out=zt, in_=pairT3[:, 4 * Q : 4 * Q + 4, JH * hf : JH * (hf + 1)])
                sq = zpool.tile([CZ, 4, JH], ZDT, tag="sq")
                nc.scalar.activation(out=sq[:, 0:3, :], in_=zt[:, 0:3, :],
                                     func=mybir.ActivationFunctionType.Square)
                nc.vector.tensor_mul(out=sq[:, 3, :], in0=zt[:, 3, :], in1=zt[:, 3, :])
                psu = pp_u.tile([128, JH], F32, tag="u")
                for q in range(4):
                    nc.tensor.matmul(psu[0:106, :], lhsT=wraw_sb[:, q], rhs=zt[:, q, :],
                                     start=(q == 0), stop=False)
                    nc.tensor.matmul(psu[0:106, :], lhsT=wsq_sb[:, q], rhs=sq[:, q, :],
                                     start=False, stop=(q == 3))
                staged = stream.tile([128, JH], SDT, tag="staged")
                nc.any.tensor_copy(out=staged, in_=psu)
                nc.gpsimd.dma_start(out=drs_d[U], in_=staged[0:106, :])
                if U == 31:
                    gather_wave(0, 32, nc.sync)
                    qTo_sb, kT_sb, v_sb, gate_sb, sraw_sb = emit_projections()
            gather_wave(32, 48, nc.sync)

            # ---- attention per head ----
            outTo_sb = persist.tile([HD, H, LC], F32R)
            for h in range(H):
                blk, off = h // 2, HP * (h % 2)
                p_sb = pstream.tile([LC, L], BF16, tag="p")
                rs = small.tile([LC, 2], F32, tag="rs")
                for jh in range(2):
                    psl = pp_u.tile([128, JH], F32, tag="u")
                    nc.tensor.matmul(psl[:LC, :JH],
                                     lhsT=qTo_sb[off : off + HD, blk, :],
                                     rhs=kT_sb[off : off + HD, blk, JH * jh : JH * (jh + 1)],
                                     start=True, stop=False)
                    nc.tensor.matmul(psl[:LC, :JH], lhsT=identb,
                                     rhs=bias_hij[:, h, JH * jh : JH * (jh + 1)],
                                     start=False, stop=True)
                    nc.scalar.activation(out=p_sb[:, JH * jh : JH * (jh + 1)],
                                         in_=psl[:LC, :JH],
                                         func=mybir.ActivationFunctionType.Exp,
                                         accum_out=rs[:, jh : jh + 1])
                rsum = small.tile([LC, 1], F32, tag="rsum")
                nc.vector.tensor_add(out=rsum, in0=rs[:, 0:1], in1=rs[:, 1:2])
                rcp = small.tile([LC, 1], F32, tag="rcp")
                nc.vector.reciprocal(out=rcp, in_=rsum)
                nc.vector.tensor_scalar_mul(out=p_sb, in0=p_sb, scalar1=rcp)
                # transpose p -> pT, then AV
                psav = pp_work.tile([HD, LC], F32, tag="work")
                for jb in range(6):
                    ptp = pp_tp.tile([128, LC], BF16, tag="tp")
                    nc.tensor.transpose(ptp, p_sb[:, 128 * jb : 128 * (jb + 1)], identb)
                    pT = pstream.tile([128, LC], BF16, tag="pT")
                    nc.any.tensor_copy(out=pT, in_=ptp)
                    nc.tensor.matmul(psav, lhsT=v_sb[:, jb, HP * h : HP * h + HD], rhs=pT,
                                     start=(jb == 0), stop=(jb == 5))
                nc.vector.tensor_copy(out=outTo_sb[:, h, :], in_=psav)

            # ---- output projection + gating + residual ----
            psy = pp_work.tile([128, 512], F32, tag="work")
            for h in range(H):
                nc.tensor.matmul(psy[:LC, :CS], lhsT=outTo_sb[:, h, :], rhs=wot_sb[:, h, :],
                                 start=(h == 0), stop=(h == H - 1))
            fin = once.tile([LC, CS], F32, tag="fin")
            nc.vector.tensor_add(out=fin, in0=psy[:LC, :CS], in1=bo_bc[:LC])
            nc.vector.tensor_mul(out=fin, in0=fin, in1=gate_sb)
            nc.vector.tensor_add(out=fin, in0=fin, in1=sraw_sb)
            nc.sync.dma_start(out=y_d[:], in_=fin)

        for _it in range(n_iter):
            if _it:
                tc.strict_bb_all_engine_barrier()
            emit_iter()

    nc.compile()
    return nc


_NC = None


def _get_nc():
    global _NC
    if _NC is None:
        _NC = build()
    return _NC


def _host_prep(single, pair, g_s, b_s, g_z, b_z, Wq, Wk, Wv, Wb, Wo, bo, Wg, bg):
    f = np.float32
    single2d = np.asarray(single, f).reshape(L, CS)
    gs = np.asarray(g_s, f)
    bs = np.asarray(b_s, f)
    gz = np.asarray(g_z, f)

    # pair-bias weights with LN-mean folded in
    gW = gz[:, None] * np.asarray(Wb, f)                 # [CZ, H]
    Wpp = gW - gW.sum(0, keepdims=True) / CZ             # [CZ, H]
    zdt = f
    if PAIR_BF16:
        import ml_dtypes
        zdt = ml_dtypes.bfloat16
    wraw = np.zeros((CZ, 4, 106), zdt)
    wsq = np.zeros((CZ, 4, 106), zdt)
    for q in range(4):
        wraw[:, q, 32 * q : 32 * q + 8] = Wpp
        wraw[:, q, 32 * q + 8] = 1.0 / CZ
        wsq[:, q, 32 * q + 9] = 1.0 / CZ

    # head-permuted projection weights (c2' = 64h + d), g_s folded, scale folded into q
    def permute_heads(Wt):                               # Wt [c1, c2] -> [c1, CP]
        out = np.zeros((CS, CP), f)
        for h in range(H):
            out[:, HP * h : HP * h + HD] = Wt[:, HD * h : HD * (h + 1)]
        return out

    sc = 1.0 / np.sqrt(HD)
    WqT = (np.asarray(Wq, f) * sc).T * gs[:, None]       # [c1, c2]
    WkT = np.asarray(Wk, f).T * gs[:, None]
    WvT = np.asarray(Wv, f).T * gs[:, None]
    WgT = np.asarray(Wg, f).T * gs[:, None]
    WoT = np.asarray(Wo, f).T                            # [c1=(h,d), c2]

    wqt = permute_heads(WqT)
    wkt = permute_heads(WkT)
    wvt = permute_heads(WvT)

    def permute_vec(vec):                                # [CS] -> [CP]
        out = np.zeros(CP, f)
        for h in range(H):
            out[HP * h : HP * h + HD] = vec[HD * h : HD * (h + 1)]
        return out

    qb = permute_vec(bs @ (np.asarray(Wq, f) * sc).T)[:, None]
    kb = permute_vec(bs @ np.asarray(Wk, f).T)[:, None]
    vb = permute_vec(bs @ np.asarray(Wv, f).T)
    gb = (bs @ np.asarray(Wg, f).T + np.asarray(bg, f)).astype(f)
    bo_v = np.asarray(bo, f)

    pair4 = np.asarray(pair, f).reshape(L, L, CZ)
    wzs = np.stack([wraw, wsq], axis=1)                  # [CZ, 2, 4, 106]
    wqkv = np.ascontiguousarray(np.stack([wqt, wkt, wvt], axis=1))  # [CS, 3, CP]
    wot_p = np.ascontiguousarray(
        WoT.reshape(H, HD, CS).transpose(1, 0, 2))       # [HD, H, CS]
    qbkb = np.concatenate([qb.reshape(4, 128).T, kb.reshape(4, 128).T], axis=1)
    bb = np.concatenate([vb, gb, bo_v]).astype(f)        # [CP + 2*CS]
    shared = dict(sing=single2d, wzs=wzs, wqkv=wqkv,
                  wgt=np.ascontiguousarray(WgT), wot=wot_p,
                  qbkb=np.ascontiguousarray(qbkb), bb=bb,
                  ident=np.eye(128, dtype=f),
                  identb=__import__('ml_dtypes').bfloat16(np.eye(LC, dtype=f)))
    in_maps = []
    for c in range(NCORES):
        i0 = LC * c
        pT = np.ascontiguousarray(
            pair4[i0 : i0 + LC].reshape(LC * L, CZ).T)   # [CZ, LC*L]
        if PAIR_BF16:
            import ml_dtypes
            pT = pT.astype(ml_dtypes.bfloat16)
        m = dict(shared)
        m["pairT"] = pT
        m["sown"] = np.ascontiguousarray(single2d[i0 : i0 + LC])
        in_maps.append(m)
    return in_maps


def kernel(**inputs) -> np.ndarray:
    nc = _get_nc()
    in_maps = _host_prep(**inputs)
    res = run_bass_kernel_spmd(nc, in_maps, list(range(NCORES)))
    out = np.empty((1, L, CS), np.float32)
    for c in range(NCORES):
        out[0, LC * c : LC * (c + 1)] = res.results[c]["y"]
    return out



# revision 19
# speedup vs baseline: 1153.8480x; 1153.8480x over previous
"""AttentionWithPairBias Trainium2 kernel, 8-way sequence-parallel over query rows.

v2: fp8-e4m3 pair stream with DoubleRow matmuls.
  - Each of the 8 cores owns 96 of the 768 query rows i.
  - Pair tensor is host-quantized to fp8 e4m3 and host-transposed per core to
    [z=128, ij] so the z-contraction maps onto the TensorE partition axis.
    LayerNorm over z is folded:  LN(z) @ (gz*Wb) = rsig_ij * (z @ W'') + const,
    W'' = gz*Wb - colsum(gz*Wb)/128.  mu and E[z^2] come from extra stationary
    columns (4/128 and 16/128 — exact fp8 powers of two; W'' is shipped as
    4*W'' so everything lands in fp8 normal range, and the 1/4 folds into the
    rsig value for free).
  - z^2 is produced on ACT/DVE/Pool (split) in fp8, and each DoubleRow matmul
    streams a (raw, sq) or (raw_i0, raw_i1) pair as the two fp8 k-halves at
    2 values/cycle — halving the dominant PE stream cost.
  - 12 i-rows pack into each PSUM bank (3 rows per 32-col strip via disjoint
    stationary columns; strips addressed by the out AP base partition), which
    cuts the PSUM->SBUF staging copies 3x vs 4-row packing.
  - The [stat, j] -> [i, h, j] remap rides a DRAM roundtrip carrying only the
    30 used rows per strip (no zero padding), in bf16.
  - q/k/v/gate projections, attention, softmax (no max-subtraction), AV and
    the output projection run per-core on its 96 rows in bf16.  The k bias is
    dropped (softmax-invariant) and the v bias is folded into bo on the host.
"""
import sys

sys.path.insert(0, "/opt/trn_rl_repo")

import numpy as np

import concourse.bacc as bacc
import concourse.bass as bass
import concourse.tile as tile
from concourse import mybir
from concourse.bass_utils import run_bass_kernel_spmd

from contextlib import ExitStack

F32 = mybir.dt.float32
BF16 = mybir.dt.bfloat16
FP8 = mybir.dt.float8e4

L = 768
CS = 384
CZ = 128
H = 8
HD = 48
HP = 64          # padded head stride in permuted c2 layout
CP = H * HP      # 512, padded c2 size for q/k/v
NCORES = 8
LC = L // NCORES  # 96 rows per core
EPS = 1e-5
JH = L // 2       # 384, half of j
RPU = 6           # i-rows per unit (DoubleRow output must sit at partition 0;
                  # 6 rows x 10 stat-cols fit the 64-col half-array limit)
NU = LC // RPU * 2  # 32 units (16 row-groups x 2 j-halves)
PB1 = 64          # wave-1 bias rows live at partitions [64:112) (engine APs
                  # may only start at partition 0 or 64)
DoubleRow = mybir.MatmulPerfMode.DoubleRow


def build(n_iter=1):
    nc = bacc.Bacc("TRN2", target_bir_lowering=False, debug=False, num_devices=NCORES)

    pairT_d = nc.declare_dram_parameter("pairT", [CZ, LC * L], FP8, isOutput=False)
    sing_d = nc.declare_dram_parameter("sing", [L, CS], BF16, isOutput=False)
    sown_d = nc.declare_dram_parameter("sown", [LC, CS], F32, isOutput=False)
    wzs_d = nc.declare_dram_parameter("wzs", [CZ, 2, 3, 2, 64], FP8, isOutput=False)
    wqkv_d = nc.declare_dram_parameter("wqkv", [CS, 3, CP], BF16, isOutput=False)
    wgt_d = nc.declare_dram_parameter("wgt", [CS, CS], BF16, isOutput=False)
    wot_d = nc.declare_dram_parameter("wot", [HD, H, CS], BF16, isOutput=False)
    qbb_d = nc.declare_dram_parameter("qbb", [128, 4], F32, isOutput=False)
    bb_d = nc.declare_dram_parameter("bb", [2 * CS], F32, isOutput=False)
    identb_d = nc.declare_dram_parameter("identb", [128, 128], BF16, isOutput=False)
    pm_d = nc.declare_dram_parameter("pm", [112, LC], BF16, isOutput=False)
    zz_d = nc.declare_dram_parameter("zz", [1, 10 * L], BF16, isOutput=False)
    y_d = nc.declare_dram_parameter("y", [LC, CS], F32, isOutput=True)
    # staged-unit scratch, one tensor per 32-col strip (whole-slab writes keep
    # the scheduler's DRAM dependency tracking exact)
    drs_d = nc.dram_tensor("drs", [NU, 60, JH], BF16)

    # [z, rowgroup(16), jhalf(2), r(6), j(384)]
    pairT5 = pairT_d[:].rearrange("z (Qh hf r j) -> z Qh hf r j", hf=2, r=RPU, j=JH)

    with tile.TileContext(nc) as tc, ExitStack() as ctx:
        singles = ctx.enter_context(tc.tile_pool(name="singles", bufs=1))
        persist = ctx.enter_context(tc.tile_pool(name="persist", bufs=1))
        arena = ctx.enter_context(tc.tile_pool(name="arena", bufs=1))
        import os
        _sb = int(os.environ.get("STREAM_BUFS", "6"))
        _zb = int(os.environ.get("Z_BUFS", "5"))
        _ub = int(os.environ.get("U_BUFS", "3"))
        _wb = int(os.environ.get("W_BUFS", "3"))
        stream = ctx.enter_context(tc.tile_pool(name="stream", bufs=_sb))
        once = ctx.enter_context(tc.tile_pool(name="once", bufs=1))
        pstream = ctx.enter_context(tc.tile_pool(name="pstream", bufs=3))
        zpool = ctx.enter_context(tc.tile_pool(name="zpool", bufs=_zb))
        small = ctx.enter_context(tc.tile_pool(name="small", bufs=4))
        pp_u = ctx.enter_context(tc.tile_pool(name="pp_u", bufs=_ub, space="PSUM"))
        pp_tp = ctx.enter_context(tc.tile_pool(name="pp_tp", bufs=2, space="PSUM"))
        pp_work = ctx.enter_context(tc.tile_pool(name="pp_work", bufs=_wb, space="PSUM"))

        # ---- constants / weights ----
        identb = singles.tile([128, 128], BF16)
        nc.scalar.dma_start(out=identb, in_=identb_d[:])
        pm_sb = singles.tile([112, LC], BF16)
        nc.scalar.dma_start(out=pm_sb, in_=pm_d[:])
        wzs_sb = singles.tile([CZ, 2, 3, 2, 64], FP8)
        nc.scalar.dma_start(out=wzs_sb, in_=wzs_d[:])
        wqkv_sb = singles.tile([128, 3, 3, CP], BF16)
        nc.scalar.dma_start(out=wqkv_sb, in_=wqkv_d[:].rearrange("(b p) w n -> p b w n", p=128))
        wgt_sb = singles.tile([128, 3, CS], BF16)
        nc.scalar.dma_start(out=wgt_sb, in_=wgt_d[:].rearrange("(b p) n -> p b n", p=128))
        wot_sb = singles.tile([HD, H, CS], BF16)
        nc.scalar.dma_start(out=wot_sb, in_=wot_d[:])
        qbb_sb = singles.tile([128, 4], F32)
        nc.scalar.dma_start(out=qbb_sb, in_=qbb_d[:])
        bb_sb = singles.tile([128, 2 * CS], F32)
        _bb = bb_d[:]
        nc.scalar.dma_start(out=bb_sb, in_=bass.AP(tensor=_bb.tensor, offset=_bb.offset,
                                                   ap=[[0, 128]] + _bb.ap))
        gb_bc = bb_sb[:, 0:CS]
        bo_bc = bb_sb[:, CS : 2 * CS]
        eps128 = singles.tile([128, 2], F32)
        nc.vector.memset(eps128[:, 0:1], EPS)
        nc.vector.memset(eps128[:, 1:2], 16.0 * EPS)
        eps_ln = eps128[:, 0:1]
        eps_z = eps128[:, 1:2]

        # ---- pair-bias landing tile (rows: i<48 at partition i, i>=48 at
        # partition i+16 — engine APs may only start at partition 0 or 64).
        # The gap rows [48:64) are never written by the waves but ARE read by
        # the pm bias-add matmul; zero them once so 0-weight x garbage can't
        # produce NaN.
        bias_hij = arena.tile([112, 10, L], BF16, tag="big")  # h=0..7 bias*4, 8=mu*4, 9=ex2*16
        _gap = bias_hij[48:64]
        nc.sync.dma_start(out=_gap,
                          in_=bass.AP(tensor=zz_d[:].tensor, offset=0,
                                      ap=[[0, 16], [1, 10 * L]]))

        def emit_rsqrt(out_ap, v_ap, tmp_pool, tag, n_newton=1):
            # out = 1/sqrt(v) on DVE only: y0 from the fast-inverse-sqrt bit
            # trick, then Newton y*(1.5 - 0.5*v*y^2).  ~0.2% max error.
            shape = [v_ap.shape[0], v_ap.shape[-1]]
            y = tmp_pool.tile([128, shape[1]], F32, tag=tag + "_y")[: shape[0]]
            t = tmp_pool.tile([128, shape[1]], F32, tag=tag + "_t")[: shape[0]]
            vu = v_ap.bitcast(mybir.dt.uint32)
            yu = y.bitcast(mybir.dt.uint32)
            nc.vector.tensor_scalar(out=yu, in0=vu, scalar1=1,
                                    op0=mybir.AluOpType.logical_shift_right)
            nc.vector.tensor_scalar(out=yu, in0=yu, scalar1=0x5F3759DF,
                                    op0=mybir.AluOpType.subtract, reverse0=True)
            for _ in range(n_newton):
                nc.vector.tensor_mul(out=t, in0=y, in1=y)
                nc.vector.tensor_mul(out=t, in0=t, in1=v_ap)
                nc.vector.tensor_scalar(out=t, in0=t, scalar1=-0.5, scalar2=1.5,
                                        op0=mybir.AluOpType.mult,
                                        op1=mybir.AluOpType.add)
                last = t is not None
                nc.vector.tensor_mul(out=(out_ap if _ == n_newton - 1 else y),
                                     in0=y, in1=t)

        def emit_iter():
            # ---- pair-bias stream ----
            rsig = persist.tile([112, L], BF16)

            def gather_wave(u0, u1, eng):
                # gather units [u0, u1) = i-rows [3*u0, 3*u1) from drs, then
                # stats -> rsig and scale this wave's bias rows in place.
                # wave0 bias rows sit at partitions [0:48), wave1 at [64:112)
                # (engine APs may only start at partition 0 or 64).
                p0 = 0 if u0 == 0 else PB1
                bias_w = bias_hij[p0 : p0 + 48, :, :]
                bias_v = bias_w.rearrange("(Qh p t) h (hf jj) -> p t hf Qh h jj",
                                          p=3, t=2, hf=2)
                drs_v = drs_d[u0:u1].rearrange(
                    "(Qh hf) (p t s) j -> p t hf Qh s j", hf=2, p=3, t=2)
                for p in range(3):
                    for t in range(2):
                        for hf in range(2):
                            eng.dma_start(out=bias_v[p, t, hf], in_=drs_v[p, t, hf])
                rs = rsig[p0 : p0 + 48, :]
                rsf_t = small.tile([128, L], F32, tag="rsf")
                rsf = rsf_t[p0 : p0 + 48]
                mu_w = bias_w[:, 8, :]
                ex2_w = bias_w[:, 9, :]
                nc.vector.tensor_mul(out=rsf, in0=mu_w, in1=mu_w)
                # rsf = (ex2 + 16eps) - mu^2
                nc.vector.scalar_tensor_tensor(out=rsf, in0=ex2_w,
                                               scalar=16.0 * EPS, in1=rsf,
                                               op0=mybir.AluOpType.add,
                                               op1=mybir.AluOpType.subtract)
                with nc.allow_low_precision(reason="rsig in bf16 is plenty for the bias scale"):
                    emit_rsqrt(rs, rsf, small, "wv")
                # scale the 8 bias rows by rsig
                for h in range(H):
                    nc.vector.tensor_mul(out=bias_w[:, h, :], in0=bias_w[:, h, :], in1=rs)

            def emit_projections():
                # ---- LayerNorm(single) ----
                s_sb = arena.tile([128, 6, CS], BF16, tag="big2")   # LN(single), i-major tiles
                so_sb = persist.tile([LC, CS], BF16)         # LN(single_own)
                x_all = once.tile([128, 6, CS], BF16, tag="ln_x")
                nc.scalar.dma_start(out=x_all, in_=sing_d[:].rearrange("(t p) n -> p t n", p=128))
                sraw_sb = persist.tile([LC, CS], F32)        # raw single_own (residual)
                nc.scalar.dma_start(out=sraw_sb, in_=sown_d[:])

                def layernorm(dst, x, rows):
                    bn = small.tile([128, 6], F32, tag="ln_bn")
                    nc.vector.bn_stats(out=bn[:rows], in_=x)
                    mv = small.tile([128, 2], F32, tag="ln_mv")
                    nc.vector.bn_aggr(out=mv[:rows], in_=bn[:rows])
                    ve = small.tile([128, 1], F32, tag="ln_ve")
                    nc.vector.tensor_scalar(out=ve[:rows], in0=mv[:rows, 1:2],
                                            scalar1=EPS, op0=mybir.AluOpType.add)
                    rstd = small.tile([128, 1], F32, tag="ln_rstd")
                    emit_rsqrt(rstd[:rows], ve[:rows], small, "lnv")
                    nc.vector.tensor_scalar(out=dst, in0=x,
                                            scalar1=mv[:rows, 0:1], scalar2=rstd[:rows],
                                            op0=mybir.AluOpType.subtract,
                                            op1=mybir.AluOpType.mult)

                for t in range(6):
                    layernorm(s_sb[:, t, :], x_all[:, t, :], 128)
                layernorm(so_sb[:], sraw_sb[:], LC)

                # ---- transposes: sT [c1, j] and sTo [c1, own-i] ----
                sT_sb = persist.tile([128, 3, L], BF16)
                for jb in range(6):
                    ptT = pp_tp.tile([128, 3, 128], BF16, tag="tp")
                    for cb in range(3):
                        nc.tensor.transpose(ptT[:, cb, :],
                                            s_sb[:, jb, 128 * cb : 128 * (cb + 1)], identb)
                    nc.scalar.activation(out=sT_sb[:, :, 128 * jb : 128 * (jb + 1)], in_=ptT,
                                         func=mybir.ActivationFunctionType.Copy)
                sTo_sb = persist.tile([128, 3, LC], BF16)
                ptT = pp_tp.tile([128, 3, 128], BF16, tag="tp")
                for cb in range(3):
                    nc.tensor.transpose(ptT[:, cb, :LC],
                                        so_sb[:, 128 * cb : 128 * (cb + 1)],
                                        identb[:LC, :LC])
                nc.scalar.activation(out=sTo_sb, in_=ptT[:, :, :LC],
                                     func=mybir.ActivationFunctionType.Copy)

                # ---- projections ----
                qTo_sb = persist.tile([128, 4, LC], BF16)      # q^T (own rows), permuted heads
                for b in range(4):
                    ps = pp_work.tile([128, 512], F32, tag="work")
                    for kb in range(3):
                        nc.tensor.matmul(ps[:, :LC], lhsT=wqkv_sb[:, kb, 0, 128 * b : 128 * (b + 1)],
                                         rhs=sTo_sb[:, kb, :], start=(kb == 0), stop=(kb == 2))
                    nc.vector.tensor_scalar_add(out=qTo_sb[:, b, :], in0=ps[:, :LC],
                                                scalar1=qbb_sb[:, b : b + 1])

                kT_sb = persist.tile([128, 4, L], BF16)        # k^T (all rows), permuted heads
                for b in range(4):
                    for jh in range(2):
                        ps = pp_work.tile([128, 512], F32, tag="work")
                        for kb in range(3):
                            nc.tensor.matmul(ps[:, :JH], lhsT=wqkv_sb[:, kb, 1, 128 * b : 128 * (b + 1)],
                                             rhs=sT_sb[:, kb, JH * jh : JH * (jh + 1)],
                                             start=(kb == 0), stop=(kb == 2))
                        nc.vector.tensor_copy(out=kT_sb[:, b, JH * jh : JH * (jh + 1)],
                                              in_=ps[:, :JH])

                v_sb = persist.tile([128, 6, CP], BF16)        # v (all rows), [j, c2-perm]
                for jb in range(6):
                    ps = pp_work.tile([128, 512], F32, tag="work")
                    for kb in range(3):
                        nc.tensor.matmul(ps, lhsT=sT_sb[:, kb, 128 * jb : 128 * (jb + 1)],
                                         rhs=wqkv_sb[:, kb, 2, :], start=(kb == 0), stop=(kb == 2))
                    nc.vector.tensor_copy(out=v_sb[:, jb, :], in_=ps)

                gate_sb = persist.tile([LC, CS], F32)
                psg = pp_work.tile([128, 512], F32, tag="work")
                for kb in range(3):
                    nc.tensor.matmul(psg[:LC, :CS], lhsT=sTo_sb[:, kb, :], rhs=wgt_sb[:, kb, :],
                                     start=(kb == 0), stop=(kb == 2))
                gtmp = once.tile([LC, CS], F32, tag="gtmp")
                nc.vector.tensor_add(out=gtmp, in0=psg[:LC, :CS], in1=gb_bc[:LC])
                # sigmoid(x) = 1/(1+exp(-x)) via the exp table (avoids a
                # sigmoid table-set load)
                nc.scalar.activation(out=gtmp, in_=gtmp,
                                     func=mybir.ActivationFunctionType.Exp,
                                     scale=-1.0)
                nc.vector.tensor_scalar(out=gtmp, in0=gtmp, scalar1=1.0,
                                        op0=mybir.AluOpType.add)
                nc.vector.reciprocal(out=gate_sb, in_=gtmp)

                return qTo_sb, kT_sb, v_sb, gate_sb, sraw_sb

            for U in range(NU):
                Qh, hf = U // 2, U % 2
                zs = zpool.tile([CZ, 2, RPU, JH], FP8, tag="zs")
                nc.sync.dma_start(out=zs[:, 0], in_=pairT5[:, Qh, hf])
                nc.scalar.activation(out=zs[:, 1, 0:3, :], in_=zs[:, 0, 0:3, :],
                                     func=mybir.ActivationFunctionType.Square)
                nc.vector.tensor_mul(out=zs[:, 1, 3:5, :], in0=zs[:, 0, 3:5, :],
                                     in1=zs[:, 0, 3:5, :])
                nc.gpsimd.tensor_mul(out=zs[:, 1, 5, :], in0=zs[:, 0, 5, :],
                                     in1=zs[:, 0, 5, :])
                psu = pp_u.tile([128, JH], F32, tag="u")
                for p in range(3):
                    t0 = 2 * p
                    nc.tensor.matmul(psu[0:64, :], lhsT=wzs_sb[:, 0, p],
                                     rhs=zs[:, 0, t0 : t0 + 2, :],
                                     start=(p == 0), stop=False, perf_mode=DoubleRow)
                    nc.tensor.matmul(psu[0:64, :], lhsT=wzs_sb[:, 1, p],
                                     rhs=zs[:, 1, t0 : t0 + 2, :],
                                     start=False, stop=(p == 2), perf_mode=DoubleRow)
                staged = stream.tile([64, JH], BF16, tag="staged")
                if U % 2 == 0:
                    nc.scalar.activation(out=staged[0:60], in_=psu[0:60],
                                         func=mybir.ActivationFunctionType.Copy)
                else:
                    nc.vector.tensor_copy(out=staged[0:60], in_=psu[0:60])
                nc.scalar.dma_start(out=drs_d[U], in_=staged[0:60, :])
                if U == NU // 2 - 1:
                    gather_wave(0, NU // 2, nc.sync)
                    qTo_sb, kT_sb, v_sb, gate_sb, sraw_sb = emit_projections()
            gather_wave(NU // 2, NU, nc.sync)

            # ---- attention per head ----
            outTo_sb = persist.tile([HD, H, LC], BF16)
            for h in range(H):
                blk, off = h // 2, HP * (h % 2)
                p_sb = pstream.tile([LC, L], BF16, tag="p")
                rs = small.tile([LC, 2], F32, tag="rs")
                for jh in range(2):
                    psl = pp_u.tile([128, JH], F32, tag="u")
                    nc.tensor.matmul(psl[:LC, :JH],
                                     lhsT=qTo_sb[off : off + HD, blk, :],
                                     rhs=kT_sb[off : off + HD, blk, JH * jh : JH * (jh + 1)],
                                     start=True, stop=False)
                    nc.tensor.matmul(psl[:LC, :JH], lhsT=pm_sb,
                                     rhs=bias_hij[:, h, JH * jh : JH * (jh + 1)],
                                     start=False, stop=True)
                    nc.scalar.activation(out=p_sb[:, JH * jh : JH * (jh + 1)],
                                         in_=psl[:LC, :JH],
                                         func=mybir.ActivationFunctionType.Exp,
                                         accum_out=rs[:, jh : jh + 1])
                rsum = small.tile([LC, 1], F32, tag="rsum")
                nc.vector.tensor_add(out=rsum, in0=rs[:, 0:1], in1=rs[:, 1:2])
                rcp = small.tile([LC, 1], F32, tag="rcp")
                nc.vector.reciprocal(out=rcp, in_=rsum)
                nc.vector.tensor_scalar_mul(out=p_sb, in0=p_sb, scalar1=rcp)
                # transpose p -> pT (merged into one PSUM bank), then AV
                ptp = pp_tp.tile([128, 6, LC], BF16, tag="tp")
                for jb in range(6):
                    nc.tensor.transpose(ptp[:, jb, :], p_sb[:, 128 * jb : 128 * (jb + 1)],
                                        identb[:LC, :LC])
                pT = pstream.tile([128, 6, LC], BF16, tag="pT")
                nc.vector.tensor_copy(out=pT, in_=ptp)
                psav = pp_work.tile([HD, LC], F32, tag="work")
                for jb in range(6):
                    nc.tensor.matmul(psav, lhsT=v_sb[:, jb, HP * h : HP * h + HD],
                                     rhs=pT[:, jb, :],
                                     start=(jb == 0), stop=(jb == 5))
                nc.scalar.activation(out=outTo_sb[:, h, :], in_=psav,
                                     func=mybir.ActivationFunctionType.Copy)

            # ---- output projection + gating + residual ----
            psy = pp_work.tile([128, 512], F32, tag="work")
            for h in range(H):
                nc.tensor.matmul(psy[:LC, :CS], lhsT=outTo_sb[:, h, :], rhs=wot_sb[:, h, :],
                                 start=(h == 0), stop=(h == H - 1))
            fin = once.tile([LC, CS], F32, tag="fin")
            nc.vector.tensor_add(out=fin, in0=psy[:LC, :CS], in1=bo_bc[:LC])
            nc.gpsimd.tensor_mul(out=fin, in0=fin, in1=gate_sb)
            nc.gpsimd.tensor_add(out=fin, in0=fin, in1=sraw_sb)
            nc.sync.dma_start(out=y_d[:], in_=fin)

        for _it in range(n_iter):
            if _it:
                tc.strict_bb_all_engine_barrier()
            emit_iter()

    nc.compile()
    return nc


_NC = None


def _get_nc():
    global _NC
    if _NC is None:
        _NC = build()
    return _NC


def _host_prep(single, pair, g_s, b_s, g_z, b_z, Wq, Wk, Wv, Wb, Wo, bo, Wg, bg):
    import ml_dtypes
    f = np.float32
    bf = ml_dtypes.bfloat16
    f8 = ml_dtypes.float8_e4m3
    single2d = np.asarray(single, f).reshape(L, CS)
    gs = np.asarray(g_s, f)
    bs = np.asarray(b_s, f)
    gz = np.asarray(g_z, f)

    # pair-bias weights with LN-mean folded in; shipped as 4*W'' in fp8 with
    # mu col 4/128 and ex2 col 16/128 (the 1/4 scale folds into rsig)
    gW = gz[:, None] * np.asarray(Wb, f)                 # [CZ, H]
    Wpp = gW - gW.sum(0, keepdims=True) / CZ             # [CZ, H]
    W4 = (4.0 * Wpp).astype(f8)
    m4 = np.float32(4.0 / CZ)
    e16 = np.float32(16.0 / CZ)
    wzs = np.zeros((CZ, 2, 3, 2, 64), f8)
    for p in range(3):
        wzs[:, 0, p, 0, 20 * p + 0 : 20 * p + 8] = W4
        wzs[:, 0, p, 0, 20 * p + 8] = m4
        wzs[:, 0, p, 1, 20 * p + 10 : 20 * p + 18] = W4
        wzs[:, 0, p, 1, 20 * p + 18] = m4
        wzs[:, 1, p, 0, 20 * p + 9] = e16
        wzs[:, 1, p, 1, 20 * p + 19] = e16

    # head-permuted projection weights (c2' = 64h + d), g_s folded, scale folded into q
    def permute_heads(Wt):                               # Wt [c1, c2] -> [c1, CP]
        out = np.zeros((CS, CP), f)
        for h in range(H):
            out[:, HP * h : HP * h + HD] = Wt[:, HD * h : HD * (h + 1)]
        return out

    sc = 1.0 / np.sqrt(HD)
    WqT = (np.asarray(Wq, f) * sc).T * gs[:, None]       # [c1, c2]
    WkT = np.asarray(Wk, f).T * gs[:, None]
    WvT = np.asarray(Wv, f).T * gs[:, None]
    WgT = np.asarray(Wg, f).T * gs[:, None]
    WoT = np.asarray(Wo, f).T                            # [c1=(h,d), c2]

    wqt = permute_heads(WqT)
    wkt = permute_heads(WkT)
    wvt = permute_heads(WvT)

    def permute_vec(vec):                                # [CS] -> [CP]
        out = np.zeros(CP, f)
        for h in range(H):
            out[HP * h : HP * h + HD] = vec[HD * h : HD * (h + 1)]
        return out

    qb = permute_vec(bs @ (np.asarray(Wq, f) * sc).T)[:, None]
    gb = (bs @ np.asarray(Wg, f).T + np.asarray(bg, f)).astype(f)
    # v bias folded into bo (sum_j p_ij = 1)
    bo_v = (np.asarray(bo, f) + (bs @ np.asarray(Wv, f).T) @ np.asarray(Wo, f).T).astype(f)

    def pm_host():
        # bias row i<48 at partition i, i>=48 at partition i+16
        pm = np.zeros((112, LC), f)
        for i in range(LC):
            pm[i if i < 48 else i + 16, i] = 1.0
        return bf(pm)

    pair4 = np.asarray(pair, f).reshape(L, L, CZ)
    wqkv = np.ascontiguousarray(
        np.stack([wqt, wkt, wvt], axis=1)).astype(bf)    # [CS, 3, CP]
    wot_p = np.ascontiguousarray(
        WoT.reshape(H, HD, CS).transpose(1, 0, 2)).astype(bf)  # [HD, H, CS]
    qbb = np.ascontiguousarray(qb.reshape(4, 128).T)     # [128, 4]
    bb = np.concatenate([gb, bo_v]).astype(f)            # [2*CS]
    shared = dict(sing=single2d.astype(bf), wzs=wzs, wqkv=wqkv,
                  wgt=np.ascontiguousarray(WgT).astype(bf), wot=wot_p,
                  qbb=qbb, bb=bb,
                  identb=bf(np.eye(128, dtype=f)),
                  pm=pm_host(), zz=np.zeros((1, 10 * L), bf))
    in_maps = []
    for c in range(NCORES):
        i0 = LC * c
        # [z, Qh(16), hf(2), r(6), j(384)]
        pT = np.ascontiguousarray(
            pair4[i0 : i0 + LC].reshape(16, RPU, 2, JH, CZ)
            .transpose(4, 0, 2, 1, 3).reshape(CZ, LC * L)).astype(f8)
        m = dict(shared)
        m["pairT"] = pT
        m["sown"] = np.ascontiguousarray(single2d[i0 : i0 + LC])
        in_maps.append(m)
    return in_maps


def kernel(**inputs) -> np.ndarray:
    nc = _get_nc()
    in_maps = _host_prep(**inputs)
    res = run_bass_kernel_spmd(nc, in_maps, list(range(NCORES)))
    out = np.empty((1, L, CS), np.float32)
    for c in range(NCORES):
        out[0, LC * c : LC * (c + 1)] = res.results[c]["y"]
    return out


# revision 28
# speedup vs baseline: 2675.8702x; 2.3191x over previous
"""AttentionWithPairBias Trainium2 kernel, 8-way sequence-parallel over query rows.

v2: fp8-e4m3 pair stream with DoubleRow matmuls.
  - Each of the 8 cores owns 96 of the 768 query rows i.
  - Pair tensor is host-quantized to fp8 e4m3 and host-transposed per core to
    [z=128, ij] so the z-contraction maps onto the TensorE partition axis.
    LayerNorm over z is folded:  LN(z) @ (gz*Wb) = rsig_ij * (z @ W'') + const,
    W'' = gz*Wb - colsum(gz*Wb)/128.  mu and E[z^2] come from extra stationary
    columns (4/128 and 16/128 — exact fp8 powers of two; W'' is shipped as
    4*W'' so everything lands in fp8 normal range, and the 1/4 folds into the
    rsig value for free).
  - z^2 is produced on ACT/DVE/Pool (split) in fp8, and each DoubleRow matmul
    streams a (raw, sq) or (raw_i0, raw_i1) pair as the two fp8 k-halves at
    2 values/cycle — halving the dominant PE stream cost.
  - 12 i-rows pack into each PSUM bank (3 rows per 32-col strip via disjoint
    stationary columns; strips addressed by the out AP base partition), which
    cuts the PSUM->SBUF staging copies 3x vs 4-row packing.
  - The [stat, j] -> [i, h, j] remap rides a DRAM roundtrip carrying only the
    30 used rows per strip (no zero padding), in bf16.
  - q/k/v/gate projections, attention, softmax (no max-subtraction), AV and
    the output projection run per-core on its 96 rows in bf16.  The k bias is
    dropped (softmax-invariant) and the v bias is folded into bo on the host.
"""
import sys

sys.path.insert(0, "/opt/trn_rl_repo")

import numpy as np

import concourse.bacc as bacc
import concourse.bass as bass
import concourse.tile as tile
from concourse import mybir
from concourse.bass_utils import run_bass_kernel_spmd

from contextlib import ExitStack

F32 = mybir.dt.float32
BF16 = mybir.dt.bfloat16
FP8 = mybir.dt.float8e4

L = 768
CS = 384
CZ = 128
H = 8
HD = 48
HP = 64          # padded head stride in permuted c2 layout
CP = H * HP      # 512, padded c2 size for q/k/v
NCORES = 8
LC = L // NCORES  # 96 rows per core
EPS = 1e-5
JH = L // 2       # 384, half of j
RPU = 6           # i-rows per unit (DoubleRow output must sit at partition 0;
                  # 6 rows x 10 stat-cols fit the 64-col half-array limit)
NU = LC // RPU * 2  # 32 units (16 row-groups x 2 j-halves)
PB1 = 64          # wave-1 bias rows live at partitions [64:112) (engine APs
                  # may only start at partition 0 or 64)
DoubleRow = mybir.MatmulPerfMode.DoubleRow


def build(n_iter=1):
    nc = bacc.Bacc("TRN2", target_bir_lowering=False, debug=False, num_devices=NCORES)

    pairT_d = nc.declare_dram_parameter("pairT", [CZ, LC * L], FP8, isOutput=False)
    sing_d = nc.declare_dram_parameter("sing", [L, CS], BF16, isOutput=False)
    sown_d = nc.declare_dram_parameter("sown", [LC, CS], F32, isOutput=False)
    wzs_d = nc.declare_dram_parameter("wzs", [CZ, 2, 3, 2, 64], FP8, isOutput=False)
    wqkv_d = nc.declare_dram_parameter("wqkv", [CS, 3, CP], BF16, isOutput=False)
    wgt_d = nc.declare_dram_parameter("wgt", [CS, CS], BF16, isOutput=False)
    wot_d = nc.declare_dram_parameter("wot", [HD, H, CS], BF16, isOutput=False)
    qbb_d = nc.declare_dram_parameter("qbb", [128, 4], F32, isOutput=False)
    bb_d = nc.declare_dram_parameter("bb", [2 * CS], F32, isOutput=False)
    identb_d = nc.declare_dram_parameter("identb", [128, 128], BF16, isOutput=False)
    pm_d = nc.declare_dram_parameter("pm", [112, LC], BF16, isOutput=False)
    zz_d = nc.declare_dram_parameter("zz", [1, 10 * L], BF16, isOutput=False)
    y_d = nc.declare_dram_parameter("y", [LC, CS], F32, isOutput=True)
    # staged-unit scratch, one tensor per 32-col strip (whole-slab writes keep
    # the scheduler's DRAM dependency tracking exact)
    drs_d = nc.dram_tensor("drs", [NU, 60, JH], BF16)

    # [z, rowgroup(16), jhalf(2), r(6), j(384)]
    pairT5 = pairT_d[:].rearrange("z (Qh hf r j) -> z Qh hf r j", hf=2, r=RPU, j=JH)

    with tile.TileContext(nc) as tc, ExitStack() as ctx:
        singles = ctx.enter_context(tc.tile_pool(name="singles", bufs=1))
        persist = ctx.enter_context(tc.tile_pool(name="persist", bufs=1))
        arena = ctx.enter_context(tc.tile_pool(name="arena", bufs=1))
        import os
        _sb = int(os.environ.get("STREAM_BUFS", "6"))
        _zb = int(os.environ.get("Z_BUFS", "5"))
        _ub = int(os.environ.get("U_BUFS", "3"))
        _wb = int(os.environ.get("W_BUFS", "3"))
        stream = ctx.enter_context(tc.tile_pool(name="stream", bufs=_sb))
        once = ctx.enter_context(tc.tile_pool(name="once", bufs=1))
        pstream = ctx.enter_context(tc.tile_pool(name="pstream", bufs=3))
        zpool = ctx.enter_context(tc.tile_pool(name="zpool", bufs=_zb))
        small = ctx.enter_context(tc.tile_pool(name="small", bufs=4))
        pp_u = ctx.enter_context(tc.tile_pool(name="pp_u", bufs=_ub, space="PSUM"))
        pp_tp = ctx.enter_context(tc.tile_pool(name="pp_tp", bufs=2, space="PSUM"))
        pp_work = ctx.enter_context(tc.tile_pool(name="pp_work", bufs=_wb, space="PSUM"))

        # ---- constants / weights ----
        identb = singles.tile([128, 128], BF16)
        nc.scalar.dma_start(out=identb, in_=identb_d[:])
        pm_sb = singles.tile([112, LC], BF16)
        nc.scalar.dma_start(out=pm_sb, in_=pm_d[:])
        wzs_sb = singles.tile([CZ, 2, 3, 2, 64], FP8)
        nc.scalar.dma_start(out=wzs_sb, in_=wzs_d[:])
        wqkv_sb = singles.tile([128, 3, 3, CP], BF16)
        nc.scalar.dma_start(out=wqkv_sb, in_=wqkv_d[:].rearrange("(b p) w n -> p b w n", p=128))
        wgt_sb = singles.tile([128, 3, CS], BF16)
        nc.scalar.dma_start(out=wgt_sb, in_=wgt_d[:].rearrange("(b p) n -> p b n", p=128))
        wot_sb = singles.tile([HD, H, CS], BF16)
        nc.scalar.dma_start(out=wot_sb, in_=wot_d[:])
        qbb_sb = singles.tile([128, 4], F32)
        nc.scalar.dma_start(out=qbb_sb, in_=qbb_d[:])
        bb_sb = singles.tile([128, 2 * CS], F32)
        _bb = bb_d[:]
        nc.scalar.dma_start(out=bb_sb, in_=bass.AP(tensor=_bb.tensor, offset=_bb.offset,
                                                   ap=[[0, 128]] + _bb.ap))
        gb_bc = bb_sb[:, 0:CS]
        bo_bc = bb_sb[:, CS : 2 * CS]
        eps128 = singles.tile([128, 2], F32)
        nc.vector.memset(eps128[:, 0:1], EPS)
        nc.vector.memset(eps128[:, 1:2], 16.0 * EPS)
        eps_ln = eps128[:, 0:1]
        eps_z = eps128[:, 1:2]

        # ---- pair-bias landing tile (rows: i<48 at partition i, i>=48 at
        # partition i+16 — engine APs may only start at partition 0 or 64).
        # The gap rows [48:64) are never written by the waves but ARE read by
        # the pm bias-add matmul; zero them once so 0-weight x garbage can't
        # produce NaN.
        bias_hij = arena.tile([112, 10, L], BF16, tag="big")  # h=0..7 bias*4, 8=mu*4, 9=ex2*16
        _gap = bias_hij[48:64]
        nc.sync.dma_start(out=_gap,
                          in_=bass.AP(tensor=zz_d[:].tensor, offset=0,
                                      ap=[[0, 16], [1, 10 * L]]))

        def emit_rsqrt(out_ap, v_ap, tmp_pool, tag, n_newton=1, eng=None):
            # out = 1/sqrt(v): y0 from the fast-inverse-sqrt bit trick, then
            # Newton y*(1.5 - 0.5*v*y^2).  ~0.2% max error.
            eng = eng or nc.vector
            shape = [v_ap.shape[0], v_ap.shape[-1]]
            b0 = v_ap.base_partition()
            y_t = tmp_pool.tile([128, shape[1]], F32, tag=tag + "_y", name=tag + "_y")
            t_t = tmp_pool.tile([128, shape[1]], F32, tag=tag + "_t", name=tag + "_t")
            y = y_t[b0 : b0 + shape[0]]
            t = t_t[b0 : b0 + shape[0]]
            vu = v_ap.bitcast(mybir.dt.uint32)
            yu = y.bitcast(mybir.dt.uint32)
            # y0 = bitcast(0x5F3759DF - (u >> 1)) via ~(u>>1) + 0x5F3759E0
            eng.tensor_scalar(out=yu, in0=vu, scalar1=1, scalar2=0xFFFFFFFF,
                              op0=mybir.AluOpType.logical_shift_right,
                              op1=mybir.AluOpType.bitwise_xor)
            eng.tensor_scalar(out=yu, in0=yu, scalar1=0x5F3759E0, scalar2=None,
                              op0=mybir.AluOpType.add)
            for it in range(n_newton):
                eng.tensor_mul(out=t, in0=y, in1=y)
                eng.tensor_mul(out=t, in0=t, in1=v_ap)
                eng.tensor_scalar(out=t, in0=t, scalar1=-0.5, scalar2=1.5,
                                  op0=mybir.AluOpType.mult,
                                  op1=mybir.AluOpType.add)
                eng.tensor_mul(out=(out_ap if it == n_newton - 1 else y),
                               in0=y, in1=t)

        def emit_iter():
            # ---- pair-bias stream ----
            rsig = persist.tile([112, L], BF16)

            def gather_wave(u0, u1, eng):
                # gather units [u0, u1) = i-rows [3*u0, 3*u1) from drs, then
                # stats -> rsig and scale this wave's bias rows in place.
                # wave0 bias rows sit at partitions [0:48), wave1 at [64:112)
                # (engine APs may only start at partition 0 or 64).
                p0 = 0 if u0 == 0 else PB1
                bias_w = bias_hij[p0 : p0 + 48, :, :]
                bias_v = bias_w.rearrange("(Qh p t) h (hf jj) -> p t hf Qh h jj",
                                          p=3, t=2, hf=2)
                drs_v = drs_d[u0:u1].rearrange(
                    "(Qh hf) (p t s) j -> p t hf Qh s j", hf=2, p=3, t=2)
                k = 0
                for p in range(3):
                    for t in range(2):
                        for hf in range(2):
                            peng = eng if k % 2 == 0 else nc.scalar
                            peng.dma_start(out=bias_v[p, t, hf], in_=drs_v[p, t, hf])
                            k += 1
                rs = rsig[p0 : p0 + 48, :]
                rsf_t = small.tile([128, L], F32, tag="rsf")
                rsf = rsf_t[p0 : p0 + 48]
                mu_w = bias_w[:, 8, :]
                ex2_w = bias_w[:, 9, :]
                # mu^2 on ACT (exp-table-resident Square), eps-fused variance,
                # sqrt on ACT, reciprocal on DVE (HW-verified numeric path);
                # split by j-half so the chain halves pipeline
                nc.scalar.activation(out=rsf, in_=mu_w,
                                     func=mybir.ActivationFunctionType.Square)
                with nc.allow_low_precision(reason="rsig in bf16 is plenty for the bias scale"):
                    for jh in range(2):
                        sl = slice(JH * jh, JH * (jh + 1))
                        nc.vector.scalar_tensor_tensor(out=rsf[:, sl], in0=ex2_w[:, sl],
                                                       scalar=16.0 * EPS, in1=rsf[:, sl],
                                                       op0=mybir.AluOpType.add,
                                                       op1=mybir.AluOpType.subtract)
                        nc.scalar.activation(out=rsf[:, sl], in_=rsf[:, sl],
                                             func=mybir.ActivationFunctionType.Sqrt)
                        nc.vector.reciprocal(out=rs[:, sl], in_=rsf[:, sl])
                # scale bias rows by rsig per (h, j-half) so each attention
                # head/jh unblocks as soon as its half is ready
                for h in range(H):
                    for jh in range(2):
                        sl = slice(JH * jh, JH * (jh + 1))
                        nc.vector.tensor_mul(out=bias_w[:, h, sl],
                                             in0=bias_w[:, h, sl], in1=rs[:, sl])

            def emit_projections():
                # ---- LayerNorm(single) ----
                s_sb = arena.tile([128, 6, CS], BF16, tag="big2")   # LN(single), i-major tiles
                so_sb = persist.tile([LC, CS], BF16)         # LN(single_own)
                x_all = once.tile([128, 6, CS], BF16, tag="ln_x")
                nc.scalar.dma_start(out=x_all, in_=sing_d[:].rearrange("(t p) n -> p t n", p=128))
                sraw_sb = persist.tile([LC, CS], F32)        # raw single_own (residual)
                nc.scalar.dma_start(out=sraw_sb, in_=sown_d[:])

                def layernorm(dst, x, rows):
                    bn = small.tile([128, 6], F32, tag="ln_bn")
                    nc.vector.bn_stats(out=bn[:rows], in_=x)
                    mv = small.tile([128, 2], F32, tag="ln_mv")
                    nc.vector.bn_aggr(out=mv[:rows], in_=bn[:rows])
                    std = small.tile([128, 1], F32, tag="ln_std")
                    nc.scalar.activation(out=std[:rows], in_=mv[:rows, 1:2],
                                         func=mybir.ActivationFunctionType.Sqrt,
                                         bias=eps_ln[:rows])
                    rstd = small.tile([128, 1], F32, tag="ln_rstd")
                    nc.vector.reciprocal(out=rstd[:rows], in_=std[:rows])
                    nc.vector.tensor_scalar(out=dst, in0=x,
                                            scalar1=mv[:rows, 0:1], scalar2=rstd[:rows],
                                            op0=mybir.AluOpType.subtract,
                                            op1=mybir.AluOpType.mult)

                for t in range(6):
                    layernorm(s_sb[:, t, :], x_all[:, t, :], 128)
                layernorm(so_sb[:], sraw_sb[:], LC)

                # ---- transposes: sT [c1, j] and sTo [c1, own-i] ----
                sT_sb = persist.tile([128, 3, L], BF16)
                for jb in range(6):
                    ptT = pp_tp.tile([128, 3, 128], BF16, tag="tp")
                    for cb in range(3):
                        nc.tensor.transpose(ptT[:, cb, :],
                                            s_sb[:, jb, 128 * cb : 128 * (cb + 1)], identb)
                    nc.scalar.activation(out=sT_sb[:, :, 128 * jb : 128 * (jb + 1)], in_=ptT,
                                         func=mybir.ActivationFunctionType.Copy)
                sTo_sb = persist.tile([128, 3, LC], BF16)
                ptT = pp_tp.tile([128, 3, 128], BF16, tag="tp")
                for cb in range(3):
                    nc.tensor.transpose(ptT[:, cb, :LC],
                                        so_sb[:, 128 * cb : 128 * (cb + 1)],
                                        identb[:LC, :LC])
                nc.scalar.activation(out=sTo_sb, in_=ptT[:, :, :LC],
                                     func=mybir.ActivationFunctionType.Copy)

                # ---- projections ----
                qTo_sb = persist.tile([128, 4, LC], BF16)      # q^T (own rows), permuted heads
                for b in range(4):
                    ps = pp_work.tile([128, 512], F32, tag="work")
                    for kb in range(3):
                        nc.tensor.matmul(ps[:, :LC], lhsT=wqkv_sb[:, kb, 0, 128 * b : 128 * (b + 1)],
                                         rhs=sTo_sb[:, kb, :], start=(kb == 0), stop=(kb == 2))
                    nc.vector.tensor_scalar_add(out=qTo_sb[:, b, :], in0=ps[:, :LC],
                                                scalar1=qbb_sb[:, b : b + 1])

                kT_sb = persist.tile([128, 4, L], BF16)        # k^T (all rows), permuted heads
                for b in range(4):
                    for jh in range(2):
                        ps = pp_work.tile([128, 512], F32, tag="work")
                        for kb in range(3):
                            nc.tensor.matmul(ps[:, :JH], lhsT=wqkv_sb[:, kb, 1, 128 * b : 128 * (b + 1)],
                                             rhs=sT_sb[:, kb, JH * jh : JH * (jh + 1)],
                                             start=(kb == 0), stop=(kb == 2))
                        nc.vector.tensor_copy(out=kT_sb[:, b, JH * jh : JH * (jh + 1)],
                                              in_=ps[:, :JH])

                v_sb = persist.tile([128, 6, CP], BF16)        # v (all rows), [j, c2-perm]
                for jb in range(6):
                    ps = pp_work.tile([128, 512], F32, tag="work")
                    for kb in range(3):
                        nc.tensor.matmul(ps, lhsT=sT_sb[:, kb, 128 * jb : 128 * (jb + 1)],
                                         rhs=wqkv_sb[:, kb, 2, :], start=(kb == 0), stop=(kb == 2))
                    nc.vector.tensor_copy(out=v_sb[:, jb, :], in_=ps)

                gate_sb = persist.tile([LC, CS], F32)
                psg = pp_work.tile([128, 512], F32, tag="work")
                for kb in range(3):
                    nc.tensor.matmul(psg[:LC, :CS], lhsT=sTo_sb[:, kb, :], rhs=wgt_sb[:, kb, :],
                                     start=(kb == 0), stop=(kb == 2))
                gtmp = once.tile([LC, CS], F32, tag="gtmp")
                nc.vector.tensor_add(out=gtmp, in0=psg[:LC, :CS], in1=gb_bc[:LC])
                nc.scalar.activation(out=gate_sb, in_=gtmp,
                                     func=mybir.ActivationFunctionType.Sigmoid)

                return qTo_sb, kT_sb, v_sb, gate_sb, sraw_sb

            zs2 = None
            for U in range(NU):
                Qh, hf = U // 2, U % 2
                if hf == 0:
                    zs2 = zpool.tile([CZ, 2, 2, RPU, JH], FP8, tag="zs")
                    (nc.sync if Qh % 2 == 0 else nc.scalar).dma_start(
                        out=zs2[:, 0], in_=pairT5[:, Qh])
                zs = zs2[:, :, hf]
                nc.scalar.activation(out=zs[:, 1, 0:3, :], in_=zs[:, 0, 0:3, :],
                                     func=mybir.ActivationFunctionType.Square)
                nc.vector.tensor_mul(out=zs[:, 1, 3:5, :], in0=zs[:, 0, 3:5, :],
                                     in1=zs[:, 0, 3:5, :])
                nc.gpsimd.tensor_mul(out=zs[:, 1, 5, :], in0=zs[:, 0, 5, :],
                                     in1=zs[:, 0, 5, :])
                psu = pp_u.tile([128, JH], F32, tag="u")
                for p in range(3):
                    t0 = 2 * p
                    nc.tensor.matmul(psu[0:64, :], lhsT=wzs_sb[:, 0, p],
                                     rhs=zs[:, 0, t0 : t0 + 2, :],
                                     start=(p == 0), stop=False, perf_mode=DoubleRow)
                    nc.tensor.matmul(psu[0:64, :], lhsT=wzs_sb[:, 1, p],
                                     rhs=zs[:, 1, t0 : t0 + 2, :],
                                     start=False, stop=(p == 2), perf_mode=DoubleRow)
                staged = stream.tile([64, JH], BF16, tag="staged")
                if U % 2 == 0:
                    nc.scalar.activation(out=staged[0:60], in_=psu[0:60],
                                         func=mybir.ActivationFunctionType.Copy)
                else:
                    nc.vector.tensor_copy(out=staged[0:60], in_=psu[0:60])
                (nc.scalar if U % 2 == 0 else nc.sync).dma_start(
                    out=drs_d[U], in_=staged[0:60, :])
                if U == NU // 2 - 1:
                    qTo_sb, kT_sb, v_sb, gate_sb, sraw_sb = emit_projections()
                elif U == 19:
                    gather_wave(0, NU // 2, nc.sync)
            gather_wave(NU // 2, NU, nc.sync)

            # ---- attention per head ----
            outTo_sb = persist.tile([HD, H, LC], BF16)
            for h in range(H):
                blk, off = h // 2, HP * (h % 2)
                p_sb = pstream.tile([LC, L], BF16, tag="p")
                rs = small.tile([LC, 2], F32, tag="rs")
                for jh in range(2):
                    psl = pp_u.tile([128, JH], F32, tag="u")
                    nc.tensor.matmul(psl[:LC, :JH],
                                     lhsT=qTo_sb[off : off + HD, blk, :],
                                     rhs=kT_sb[off : off + HD, blk, JH * jh : JH * (jh + 1)],
                                     start=True, stop=False)
                    nc.tensor.matmul(psl[:LC, :JH], lhsT=pm_sb,
                                     rhs=bias_hij[:, h, JH * jh : JH * (jh + 1)],
                                     start=False, stop=True)
                    nc.scalar.activation(out=p_sb[:, JH * jh : JH * (jh + 1)],
                                         in_=psl[:LC, :JH],
                                         func=mybir.ActivationFunctionType.Exp,
                                         accum_out=rs[:, jh : jh + 1])
                rsum = small.tile([LC, 1], F32, tag="rsum")
                nc.vector.tensor_add(out=rsum, in0=rs[:, 0:1], in1=rs[:, 1:2])
                rcp = small.tile([LC, 1], F32, tag="rcp")
                nc.vector.reciprocal(out=rcp, in_=rsum)
                nc.vector.tensor_scalar_mul(out=p_sb, in0=p_sb, scalar1=rcp)
                # transpose p -> pT (merged into one PSUM bank), then AV
                ptp = pp_tp.tile([128, 6, LC], BF16, tag="tp")
                for jb in range(6):
                    nc.tensor.transpose(ptp[:, jb, :], p_sb[:, 128 * jb : 128 * (jb + 1)],
                                        identb[:LC, :LC])
                pT = pstream.tile([128, 6, LC], BF16, tag="pT")
                nc.vector.tensor_copy(out=pT, in_=ptp)
                psav = pp_work.tile([HD, LC], F32, tag="work")
                for jb in range(6):
                    nc.tensor.matmul(psav, lhsT=v_sb[:, jb, HP * h : HP * h + HD],
                                     rhs=pT[:, jb, :],
                                     start=(jb == 0), stop=(jb == 5))
                nc.scalar.activation(out=outTo_sb[:, h, :], in_=psav,
                                     func=mybir.ActivationFunctionType.Copy)

            # ---- output projection + gating + residual ----
            psy = pp_work.tile([128, 512], F32, tag="work")
            for h in range(H):
                nc.tensor.matmul(psy[:LC, :CS], lhsT=outTo_sb[:, h, :], rhs=wot_sb[:, h, :],
                                 start=(h == 0), stop=(h == H - 1))
            fin = once.tile([LC, CS], F32, tag="fin")
            nc.vector.tensor_add(out=fin, in0=psy[:LC, :CS], in1=bo_bc[:LC])
            nc.vector.tensor_mul(out=fin, in0=fin, in1=gate_sb)
            nc.vector.tensor_add(out=fin, in0=fin, in1=sraw_sb)
            nc.sync.dma_start(out=y_d[:], in_=fin)

        for _it in range(n_iter):
            if _it:
                tc.strict_bb_all_engine_barrier()
            emit_iter()

    nc.compile()
    return nc


_NC = None


def _get_nc():
    global _NC
    if _NC is None:
        _NC = build()
    return _NC


def _host_prep(single, pair, g_s, b_s, g_z, b_z, Wq, Wk, Wv, Wb, Wo, bo, Wg, bg):
    import ml_dtypes
    f = np.float32
    bf = ml_dtypes.bfloat16
    f8 = ml_dtypes.float8_e4m3
    single2d = np.asarray(single, f).reshape(L, CS)
    gs = np.asarray(g_s, f)
    bs = np.asarray(b_s, f)
    gz = np.asarray(g_z, f)

    # pair-bias weights with LN-mean folded in; shipped as 4*W'' in fp8 with
    # mu col 4/128 and ex2 col 16/128 (the 1/4 scale folds into rsig)
    gW = gz[:, None] * np.asarray(Wb, f)                 # [CZ, H]
    Wpp = gW - gW.sum(0, keepdims=True) / CZ             # [CZ, H]
    W4 = (4.0 * Wpp).astype(f8)
    m4 = np.float32(4.0 / CZ)
    e16 = np.float32(16.0 / CZ)
    wzs = np.zeros((CZ, 2, 3, 2, 64), f8)
    for p in range(3):
        wzs[:, 0, p, 0, 20 * p + 0 : 20 * p + 8] = W4
        wzs[:, 0, p, 0, 20 * p + 8] = m4
        wzs[:, 0, p, 1, 20 * p + 10 : 20 * p + 18] = W4
        wzs[:, 0, p, 1, 20 * p + 18] = m4
        wzs[:, 1, p, 0, 20 * p + 9] = e16
        wzs[:, 1, p, 1, 20 * p + 19] = e16

    # head-permuted projection weights (c2' = 64h + d), g_s folded, scale folded into q
    def permute_heads(Wt):                               # Wt [c1, c2] -> [c1, CP]
        out = np.zeros((CS, CP), f)
        for h in range(H):
            out[:, HP * h : HP * h + HD] = Wt[:, HD * h : HD * (h + 1)]
        return out

    sc = 1.0 / np.sqrt(HD)
    WqT = (np.asarray(Wq, f) * sc).T * gs[:, None]       # [c1, c2]
    WkT = np.asarray(Wk, f).T * gs[:, None]
    WvT = np.asarray(Wv, f).T * gs[:, None]
    WgT = np.asarray(Wg, f).T * gs[:, None]
    WoT = np.asarray(Wo, f).T                            # [c1=(h,d), c2]

    wqt = permute_heads(WqT)
    wkt = permute_heads(WkT)
    wvt = permute_heads(WvT)

    def permute_vec(vec):                                # [CS] -> [CP]
        out = np.zeros(CP, f)
        for h in range(H):
            out[HP * h : HP * h + HD] = vec[HD * h : HD * (h + 1)]
        return out

    qb = permute_vec(bs @ (np.asarray(Wq, f) * sc).T)[:, None]
    gb = (bs @ np.asarray(Wg, f).T + np.asarray(bg, f)).astype(f)
    # v bias folded into bo (sum_j p_ij = 1)
    bo_v = (np.asarray(bo, f) + (bs @ np.asarray(Wv, f).T) @ np.asarray(Wo, f).T).astype(f)

    def pm_host():
        # bias row i<48 at partition i, i>=48 at partition i+16
        pm = np.zeros((112, LC), f)
        for i in range(LC):
            pm[i if i < 48 else i + 16, i] = 1.0
        return bf(pm)

    pair4 = np.asarray(pair, f).reshape(L, L, CZ)
    wqkv = np.ascontiguousarray(
        np.stack([wqt, wkt, wvt], axis=1)).astype(bf)    # [CS, 3, CP]
    wot_p = np.ascontiguousarray(
        WoT.reshape(H, HD, CS).transpose(1, 0, 2)).astype(bf)  # [HD, H, CS]
    qbb = np.ascontiguousarray(qb.reshape(4, 128).T)     # [128, 4]
    bb = np.concatenate([gb, bo_v]).astype(f)            # [2*CS]
    shared = dict(sing=single2d.astype(bf), wzs=wzs, wqkv=wqkv,
                  wgt=np.ascontiguousarray(WgT).astype(bf), wot=wot_p,
                  qbb=qbb, bb=bb,
                  identb=bf(np.eye(128, dtype=f)),
                  pm=pm_host(), zz=np.zeros((1, 10 * L), bf))
    in_maps = []
    for c in range(NCORES):
        i0 = LC * c
        # [z, Qh(16), hf(2), r(6), j(384)]
        pT = np.ascontiguousarray(
            pair4[i0 : i0 + LC].reshape(16, RPU, 2, JH, CZ)
            .transpose(4, 0, 2, 1, 3).reshape(CZ, LC * L)).astype(f8)
        m = dict(shared)
        m["pairT"] = pT
        m["sown"] = np.ascontiguousarray(single2d[i0 : i0 + LC])
        in_maps.append(m)
    return in_maps


def kernel(**inputs) -> np.ndarray:
    nc = _get_nc()
    in_maps = _host_prep(**inputs)
    res = run_bass_kernel_spmd(nc, in_maps, list(range(NCORES)))
    out = np.empty((1, L, CS), np.float32)
    for c in range(NCORES):
        out[0, LC * c : LC * (c + 1)] = res.results[c]["y"]
    return out
